# revision 1
# baseline (speedup 1.0000x reference)
"""Trainium2 Bass kernel for nn_Critic (8-agent attention critic).

Strategy: data-parallel over batch (axis 1) across 8 NeuronCores.
BatchNorm (training-mode, per-agent) is algebraically folded into the
first-layer weights on the host: the batch mean/var depend only on the
inputs, so  BN(x) @ W + b == x @ (scale*W) + (b + off @ W).
The per-sample 8x8 cross-agent attention (no softmax) runs on-device:
  P_ij   = q_i * k_j                      (DVE, elementwise bf16)
  G_ij   = kron(I4, ones(32,32)).T @ P_ij (TensorE: per-head sum of P over
           d', broadcast across d -> alpha_ij replicated, in PSUM)
  m_ij   = G_ij * v_j                     (DVE)
  T_i    = sum_{j != i} m_ij              (TensorE identity-matmul PSUM accum)
Final argmax-gather is done with a host-precomputed onehot of
argmax(action) (exact same argmax the reference takes), applied on-device
as a mask + ones-matmul column sum.

Self-contained: hardcodes shapes; needs only /opt/trn_rl_repo on sys.path.
"""
import sys

sys.path.insert(0, "/opt/trn_rl_repo")

import numpy as np
import ml_dtypes

import concourse.bass as bass
import concourse.mybir as mybir
import concourse.tile as tile
from concourse import bacc
from concourse.alu_op_type import AluOpType
from concourse.bass_utils import run_bass_kernel_spmd

BF16 = mybir.dt.bfloat16
F32 = mybir.dt.float32
AF = mybir.ActivationFunctionType

A, B, OBS, ACT, E, H = 8, 32768, 128, 32, 128, 4
D = E // H
NCORES = 8
EPS = 1e-5
SLOPE = 0.01  # leaky relu
POOL_TT = False  # neuronxcc rejects generic TT ops on the Pool engine


def build_nc(Bs, CH):
    """Build the per-core SPMD program. Bs = batch shard per core, CH = chunk."""
    NCH = Bs // CH
    nc = bacc.Bacc(None, target_bir_lowering=False, debug=False)

    obs_e = nc.declare_dram_parameter("obs_T", [A, OBS, Bs], BF16, isOutput=False)
    act_e = nc.declare_dram_parameter("act_T", [A, ACT, Bs], BF16, isOutput=False)
    oh_e = nc.declare_dram_parameter("oh_T", [A, ACT, Bs], BF16, isOutput=False)
    wgo_e = nc.declare_dram_parameter("wgo", [A, OBS, E], BF16, isOutput=False)
    wga_e = nc.declare_dram_parameter("wga", [A, ACT, E], BF16, isOutput=False)
    ws_e = nc.declare_dram_parameter("ws", [A, OBS, E], BF16, isOutput=False)
    wq_e = nc.declare_dram_parameter("wq", [E, E], BF16, isOutput=False)
    wk_e = nc.declare_dram_parameter("wk", [E, E], BF16, isOutput=False)
    wv_e = nc.declare_dram_parameter("wv", [E, E], BF16, isOutput=False)
    wf1x_e = nc.declare_dram_parameter("wf1x", [A, E, E], BF16, isOutput=False)
    wf1s_e = nc.declare_dram_parameter("wf1s", [A, E, E], BF16, isOutput=False)
    wf2_e = nc.declare_dram_parameter("wf2", [A, E, ACT], BF16, isOutput=False)
    delta_e = nc.declare_dram_parameter("delta", [E, E], BF16, isOutput=False)
    ident_e = nc.declare_dram_parameter("ident", [E, E], BF16, isOutput=False)
    bg_e = nc.declare_dram_parameter("bg_t", [E, A], F32, isOutput=False)
    bs_e = nc.declare_dram_parameter("bs_t", [E, A], F32, isOutput=False)
    bh1_e = nc.declare_dram_parameter("bh1_t", [E, A], F32, isOutput=False)
    bf2_e = nc.declare_dram_parameter("bf2_t", [ACT, A], F32, isOutput=False)
    rowsel_e = nc.declare_dram_parameter("rowsel", [ACT, A * A], BF16, isOutput=False)
    out_e = nc.declare_dram_parameter("out", [A, Bs], F32, isOutput=True)

    with tile.TileContext(nc) as tc:
        with (
            tc.tile_pool(name="wpool", bufs=1) as wp,
            tc.tile_pool(name="xo", bufs=4) as xo_p,
            tc.tile_pool(name="xa", bufs=4) as xa_p,
            tc.tile_pool(name="oh", bufs=2) as oh_p,
            tc.tile_pool(name="store", bufs=2) as st_p,
            tc.tile_pool(name="eraw", bufs=4) as eraw_p,
            tc.tile_pool(name="et", bufs=3) as et_p,
            tc.tile_pool(name="pp", bufs=3) as pp_p,
            tc.tile_pool(name="lru", bufs=3) as lru_p,
            tc.tile_pool(name="mm", bufs=3) as mm_p,
            tc.tile_pool(name="h1", bufs=3) as h1_p,
            tc.tile_pool(name="f2", bufs=4) as f2_p,
            tc.tile_pool(name="orow", bufs=3) as orow_p,
            tc.tile_pool(name="ps_mm", bufs=3, space="PSUM") as ps_mm,
            tc.tile_pool(name="ps_g", bufs=2, space="PSUM") as ps_g,
            tc.tile_pool(name="ps_row", bufs=1, space="PSUM") as ps_row,
        ):
            # ---- load weights once ----
            wgo_t = wp.tile([OBS, A * E], BF16)
            wga_t = wp.tile([ACT, A * E], BF16)
            ws_t = wp.tile([OBS, A * E], BF16)
            wq_t = wp.tile([E, E], BF16)
            wk_t = wp.tile([E, E], BF16)
            wv_t = wp.tile([E, E], BF16)
            wf1x_t = wp.tile([E, A * E], BF16)
            wf1s_t = wp.tile([E, A * E], BF16)
            wf2_t = wp.tile([E, A * ACT], BF16)
            delta_t = wp.tile([E, E], BF16)
            ident_t = wp.tile([E, E], BF16)
            bg_t = wp.tile([E, A], F32)
            bs_t = wp.tile([E, A], F32)
            bh1_t = wp.tile([E, A], F32)
            bf2_t = wp.tile([ACT, A], F32)
            ones_t = wp.tile([ACT, 1], BF16)
            rowsel_t = wp.tile([ACT, A * A], BF16)
            c001_t = wp.tile([E, 512], BF16)

            for a in range(A):
                nc.sync.dma_start(wgo_t[:, a * E:(a + 1) * E], wgo_e[a])
                nc.sync.dma_start(wga_t[:, a * E:(a + 1) * E], wga_e[a])
                nc.sync.dma_start(ws_t[:, a * E:(a + 1) * E], ws_e[a])
                nc.sync.dma_start(wf1x_t[:, a * E:(a + 1) * E], wf1x_e[a])
                nc.sync.dma_start(wf1s_t[:, a * E:(a + 1) * E], wf1s_e[a])
                nc.sync.dma_start(wf2_t[:, a * ACT:(a + 1) * ACT], wf2_e[a])
            nc.sync.dma_start(wq_t[:], wq_e[:])
            nc.sync.dma_start(wk_t[:], wk_e[:])
            nc.sync.dma_start(wv_t[:], wv_e[:])
            nc.sync.dma_start(delta_t[:], delta_e[:])
            nc.sync.dma_start(ident_t[:], ident_e[:])
            nc.sync.dma_start(bg_t[:], bg_e[:])
            nc.sync.dma_start(bs_t[:], bs_e[:])
            nc.sync.dma_start(bh1_t[:], bh1_e[:])
            nc.sync.dma_start(bf2_t[:], bf2_e[:])
            nc.sync.dma_start(rowsel_t[:], rowsel_e[:])
            nc.vector.memset(ones_t[:], 1.0)
            nc.vector.memset(c001_t[:], SLOPE)

            def lrelu_to(dst, src_t, pool_scratch):
                # max(0.01*x, x) in one fused DVE op (1x mode, but a split
                # tensor_scalar+tensor_tensor pair measured slower overall)
                nc.vector.scalar_tensor_tensor(dst, src_t[:], SLOPE,
                                               src_t[:], AluOpType.mult,
                                               AluOpType.max)

            for ch in range(NCH):
                c0 = ch * CH
                s_st = st_p.tile([E, A * CH], BF16, tag="s_st")
                q_st = st_p.tile([E, A * CH], BF16, tag="q_st")
                k_st = st_p.tile([E, A * CH], BF16, tag="k_st")
                v_st = st_p.tile([E, A * CH], BF16, tag="v_st")
                oh_t = oh_p.tile([ACT, A * CH], BF16)

                def stage_a1(a):
                    # load x; e = lrelu(x @ Wg' + bg'); s = lrelu(x_o @ Ws'+bs')
                    asl = slice(a * CH, (a + 1) * CH)
                    x_o = xo_p.tile([OBS, CH], BF16)
                    x_a = xa_p.tile([ACT, CH], BF16)
                    nc.sync.dma_start(x_o[:], obs_e[a][:, c0:c0 + CH])
                    nc.sync.dma_start(x_a[:], act_e[a][:, c0:c0 + CH])
                    nc.sync.dma_start(oh_t[:, asl], oh_e[a][:, c0:c0 + CH])
                    pe = ps_mm.tile([E, CH], F32, tag="ps")
                    nc.tensor.matmul(pe[:], wgo_t[:, a * E:(a + 1) * E], x_o[:],
                                     start=True, stop=False)
                    nc.tensor.matmul(pe[:], wga_t[:, a * E:(a + 1) * E], x_a[:],
                                     start=False, stop=True)
                    e_raw = eraw_p.tile([E, CH], BF16, tag="raw")
                    nc.scalar.activation(e_raw[:], pe[:], AF.Identity,
                                         bias=bg_t[:, a:a + 1])
                    e_t = et_p.tile([E, CH], BF16, tag="e_t")
                    lrelu_to(e_t[:], e_raw, et_p)
                    psm = ps_mm.tile([E, CH], F32, tag="ps")
                    nc.tensor.matmul(psm[:], ws_t[:, a * E:(a + 1) * E], x_o[:],
                                     start=True, stop=True)
                    s_raw = eraw_p.tile([E, CH], BF16, tag="raw")
                    nc.scalar.activation(s_raw[:], psm[:], AF.Identity,
                                         bias=bs_t[:, a:a + 1])
                    lrelu_to(s_st[:, asl], s_raw, et_p)
                    return e_t

                def stage_a2(a, e_t):
                    # q, k, v projections from e
                    asl = slice(a * CH, (a + 1) * CH)
                    pq = ps_mm.tile([E, CH], F32, tag="ps")
                    nc.tensor.matmul(pq[:], wq_t[:], e_t[:], start=True, stop=True)
                    nc.scalar.activation(q_st[:, asl], pq[:], AF.Identity)
                    pk = ps_mm.tile([E, CH], F32, tag="ps")
                    nc.tensor.matmul(pk[:], wk_t[:], e_t[:], start=True, stop=True)
                    nc.scalar.activation(k_st[:, asl], pk[:], AF.Identity)
                    pv = ps_mm.tile([E, CH], F32, tag="ps")
                    nc.tensor.matmul(pv[:], wv_t[:], e_t[:], start=True, stop=True)
                    v_raw = eraw_p.tile([E, CH], BF16, tag="raw")
                    nc.scalar.activation(v_raw[:], pv[:], AF.Identity)
                    lrelu_to(v_st[:, asl], v_raw, et_p)

                pend_e = None
                for a in range(A):
                    e_t = stage_a1(a)
                    if pend_e is not None:
                        stage_a2(a - 1, pend_e)
                    pend_e = e_t
                stage_a2(A - 1, pend_e)

                def stage_i0(i):
                    # P_j = q_i * k_j (batched over 4 j per op); on even i the
                    # second half goes to the Pool engine to unload DVE.
                    isl = slice(i * CH, (i + 1) * CH)
                    P_all = pp_p.tile([E, A * CH], BF16)
                    for half in range(2):
                        hsl = slice(half * 4 * CH, (half + 1) * 4 * CH)
                        q_rep = q_st[:, None, isl].broadcast_to([E, 4, CH])
                        nc.vector.tensor_tensor(
                            P_all[:, hsl].rearrange("p (j b) -> p j b", j=4),
                            q_rep,
                            k_st[:, hsl].rearrange("p (j b) -> p j b", j=4),
                            AluOpType.mult)
                    return P_all

                def stage_i1(i, P_all):
                    # G_j = per-head sum of P over d', broadcast (alpha*sqrt(D));
                    # m_j = G_j * v_j. Quad 0: DVE reads G from PSUM; quads 1-3:
                    # ScalarE evacuates G, DVE multiplies from SBUF (2x mode).
                    # j == i is skipped entirely (masked out of the f1 sum).
                    m_all = mm_p.tile([E, A * CH], BF16)
                    for quad in range(4):
                        js = [j for j in (2 * quad, 2 * quad + 1) if j != i]
                        w = len(js)
                        pG = ps_g.tile([E, 2 * CH], F32)
                        for jj, j in enumerate(js):
                            nc.tensor.matmul(pG[:, jj * CH:(jj + 1) * CH],
                                             delta_t[:],
                                             P_all[:, j * CH:(j + 1) * CH],
                                             start=True, stop=True)
                        if w == 2:
                            dst = m_all[:, js[0] * CH:(js[1] + 1) * CH]
                            vsrc = v_st[:, js[0] * CH:(js[1] + 1) * CH]
                        else:
                            dst = m_all[:, js[0] * CH:(js[0] + 1) * CH]
                            vsrc = v_st[:, js[0] * CH:(js[0] + 1) * CH]
                        if quad > 0:
                            g_sb = lru_p.tile([E, 2 * CH], BF16, tag="g_sb")
                            nc.scalar.activation(g_sb[:, :w * CH],
                                                 pG[:, :w * CH], AF.Identity)
                            nc.vector.tensor_tensor(dst, g_sb[:, :w * CH],
                                                    vsrc, AluOpType.mult)
                        else:
                            nc.vector.tensor_tensor(dst, pG[:, :w * CH],
                                                    vsrc, AluOpType.mult)
                    return m_all

                def stage_i2(i, m_all):
                    # h1 psum accumulates f1 over the 7 m_j blocks (f1 is
                    # linear in xi, so no separate T accumulation) + s part.
                    isl = slice(i * CH, (i + 1) * CH)
                    ph = ps_mm.tile([E, CH], F32, tag="ps")
                    others = [j for j in range(A) if j != i]
                    for nj, j in enumerate(others):
                        nc.tensor.matmul(ph[:], wf1x_t[:, i * E:(i + 1) * E],
                                         m_all[:, j * CH:(j + 1) * CH],
                                         start=(nj == 0), stop=False)
                    nc.tensor.matmul(ph[:], wf1s_t[:, i * E:(i + 1) * E],
                                     s_st[:, isl], start=False, stop=True)
                    h1_raw = h1_p.tile([E, CH], BF16, tag="h1_raw")
                    nc.scalar.activation(h1_raw[:], ph[:], AF.Identity,
                                         bias=bh1_t[:, i:i + 1])
                    h1_t = h1_p.tile([E, CH], BF16, tag="h1_t")
                    lrelu_to(h1_t[:], h1_raw, h1_p)
                    # f2 + bias, mask by onehot, column-sum via ones matmul
                    pf = ps_mm.tile([ACT, CH], F32, tag="ps")
                    nc.tensor.matmul(pf[:], wf2_t[:, i * ACT:(i + 1) * ACT],
                                     h1_t[:], start=True, stop=True)
                    f2_sb = f2_p.tile([ACT, CH], BF16, tag="f2sb")
                    nc.scalar.activation(f2_sb[:], pf[:], AF.Identity,
                                         bias=bf2_t[:, i:i + 1])
                    msk = f2_p.tile([ACT, CH], BF16, tag="msk")
                    if POOL_TT:
                        nc.gpsimd.tensor_tensor(msk[:], f2_sb[:], oh_t[:, isl],
                                                AluOpType.mult)
                    else:
                        nc.vector.tensor_tensor(msk[:], f2_sb[:], oh_t[:, isl],
                                                AluOpType.mult)
                    nc.tensor.matmul(prow_acc[:],
                                     rowsel_t[:, i * A:(i + 1) * A], msk[:],
                                     start=(i == 0), stop=(i == A - 1))

                prow_acc = ps_row.tile([A, CH], F32)
                pend = {}
                for t in range(A + 2):
                    if t < A:
                        pend[("P", t)] = stage_i0(t)
                    if 1 <= t < A + 1:
                        pend[("m", t - 1)] = stage_i1(t - 1, pend.pop(("P", t - 1)))
                    if t >= 2:
                        stage_i2(t - 2, pend.pop(("m", t - 2)))
                orow8 = orow_p.tile([A, CH], F32)
                nc.scalar.activation(orow8[:], prow_acc[:], AF.Identity)
                nc.sync.dma_start(out_e[:, c0:c0 + CH], orow8[:])

    nc.compile()
    return nc


def _rowsel():
    # lhsT block i is [ACT, A] with column i all-ones: the ones-matmul then
    # lands agent i's column sum in PSUM partition row i (accumulated over i).
    rs = np.zeros((ACT, A * A), np.float32)
    for i in range(A):
        rs[:, i * A + i] = 1.0
    return rs


def _fold_weights(inputs):
    """Fold training-mode BN into first-layer weights; pack for device."""
    f32 = np.float32
    obs = np.asarray(inputs["observation_vector"], f32)
    act = np.asarray(inputs["action_vector"], f32)
    g_gamma = np.asarray(inputs["g_gamma"], np.float64)
    g_beta = np.asarray(inputs["g_beta"], np.float64)
    Wg = np.asarray(inputs["Wg"], np.float64)
    bg = np.asarray(inputs["bg"], np.float64)
    s_gamma = np.asarray(inputs["s_gamma"], np.float64)
    s_beta = np.asarray(inputs["s_beta"], np.float64)
    Ws = np.asarray(inputs["Ws"], np.float64)
    bs = np.asarray(inputs["bs"], np.float64)

    mean_o = obs.mean(axis=1, dtype=np.float64)
    var_o = obs.var(axis=1, dtype=np.float64)
    mean_a = act.mean(axis=1, dtype=np.float64)
    var_a = act.var(axis=1, dtype=np.float64)
    mean = np.concatenate([mean_o, mean_a], axis=1)  # [A, OBS+ACT]
    var = np.concatenate([var_o, var_a], axis=1)

    sc_g = g_gamma / np.sqrt(var + EPS)
    off_g = g_beta - mean * sc_g
    Wg_f = Wg * sc_g[:, :, None]
    bg_f = bg + np.einsum("af,afe->ae", off_g, Wg)

    sc_s = s_gamma / np.sqrt(var_o + EPS)
    off_s = s_beta - mean_o * sc_s
    Ws_f = Ws * sc_s[:, :, None]
    bs_f = bs + np.einsum("af,afe->ae", off_s, Ws)

    bf16 = ml_dtypes.bfloat16
    Wq = np.asarray(inputs["Wq"], f32)
    Wk = np.asarray(inputs["Wk"], f32)
    Wv = np.asarray(inputs["Wv"], f32)
    Wf1 = np.asarray(inputs["Wf1"], np.float64)
    Wf2 = np.asarray(inputs["Wf2"], f32)

    w = {
        "wgo": Wg_f[:, :OBS, :].astype(f32).astype(bf16),
        "wga": Wg_f[:, OBS:, :].astype(f32).astype(bf16),
        "ws": Ws_f.astype(f32).astype(bf16),
        "wq": np.ascontiguousarray(Wq.transpose(1, 0, 2).reshape(E, E)).astype(bf16),
        "wk": np.ascontiguousarray(Wk.transpose(1, 0, 2).reshape(E, E)).astype(bf16),
        "wv": np.ascontiguousarray(Wv.transpose(1, 0, 2).reshape(E, E)).astype(bf16),
        "wf1x": (Wf1[:, :E, :] / np.sqrt(D)).astype(f32).astype(bf16),
        "wf1s": Wf1[:, E:, :].astype(f32).astype(bf16),
        "wf2": Wf2.astype(bf16),
        "delta": np.kron(np.eye(H, dtype=f32),
                         np.ones((D, D), f32)).astype(bf16),
        "ident": np.eye(E, dtype=f32).astype(bf16),
        "bg_t": np.ascontiguousarray(bg_f.T.astype(f32)),
        "bs_t": np.ascontiguousarray(bs_f.T.astype(f32)),
        "bh1_t": np.ascontiguousarray(np.asarray(inputs["bf1"], f32).T),
        "bf2_t": np.ascontiguousarray(np.asarray(inputs["bf2"], f32).T),
        "rowsel": _rowsel().astype(bf16),
    }
    return w, obs, act


def make_in_maps(inputs, Bs):
    w, obs, act = _fold_weights(inputs)
    bf16 = ml_dtypes.bfloat16
    ids = np.argmax(np.asarray(inputs["action_vector"], np.float32), axis=2)  # [A,B]
    oh = (ids[:, :, None] == np.arange(ACT)[None, None, :]).astype(np.float32)

    in_maps = []
    for c in range(NCORES):
        sl = slice(c * Bs, (c + 1) * Bs)
        m = dict(w)
        m["obs_T"] = np.ascontiguousarray(
            obs[:, sl, :].transpose(0, 2, 1)).astype(bf16)
        m["act_T"] = np.ascontiguousarray(
            act[:, sl, :].transpose(0, 2, 1)).astype(bf16)
        m["oh_T"] = np.ascontiguousarray(
            oh[:, sl, :].transpose(0, 2, 1)).astype(bf16)
        in_maps.append(m)
    return in_maps


_NC_CACHE = {}


def run(inputs, trace=False, **kw):
    Bs = B // NCORES
    in_maps = make_in_maps(inputs, Bs)
    key = (Bs, 512)
    if key not in _NC_CACHE:
        _NC_CACHE[key] = build_nc(Bs, 512)
    nc = _NC_CACHE[key]
    res = run_bass_kernel_spmd(nc, in_maps, core_ids=list(range(NCORES)),
                               trace=trace, **kw)
    outs = [r["out"] for r in res.results]  # each [A, Bs] f32
    full = np.concatenate(outs, axis=1)  # [A, B]
    return full.reshape(A, B, 1).astype(np.float32), res


def kernel(**inputs):
    out, _ = run(inputs, trace=False)
    return out


if __name__ == "__main__":
    rng = np.random.default_rng(0)
    print("kernel.py loaded")

